# revision 30
# baseline (speedup 1.0000x reference)
"""Distributed Trainium2 Bass kernel for multi-head attention.

Reference computation (B=4, S=2048, D=1024, H=16 heads, HD=64):
    q = heads(Q @ Wq + bq + Q_lev)
    k = heads(K @ Wk + bk + K_lev)
    v = heads(V @ Wv + bv + V_lev)
    out = softmax(q k^T / sqrt(HD)) v  -> merge heads -> @ Wo + bo

Sharding: 8 cores = 4 batches x 2 query-halves (1024 queries each).
Each core computes its [1024, 1024] output slice end-to-end with zero
collectives; the K/V projections are recomputed by both cores of a
batch pair (cheaper than any 2-rank collective on this chip).

Device-side layout strategy (everything feature-major / pre-transposed
on the host so no on-chip transposes are needed):
  qT   [D, Sq]  = Wq.T @ Q.T   (+ bq + Q_lev, folded on host into qlevT)
  kT   [D, S]   = Wk.T @ K.T
  v    [S, D]   = V @ Wv       (stationary = V.T, moving = Wv)
  scoresT[keys, q] = kT_h.T @ qT_h        (contract over HD=64)
  probsT = exp(scoresT / 8)               (no max subtraction: scores are
                                           N(0,~2) so exp stays < ~1e6)
  ctxT_aug[65, q] = v_aug_h.T @ probsT    (v_aug has a 65th ones column,
                                           so row 64 = softmax denominator)
  ctxT = ctxT_aug[:64] * (1/denominator)  (batched reciprocal + block-diag
                                           ones matmul to broadcast 1/sum
                                           across the 64 head-dim partitions)
  out[q, D] = ctxT.T @ Wo (+ bo)

Matmuls run in bf16 (f32 PSUM accumulation). The two K=64 scores
matmuls of a head pair run concurrently in PE row halves (tile_position
auto-derived from base partitions 0/64) and write the two banks of one
[128, 1024] PSUM tile so a single wide ACT exp serves both heads.

Scheduling: the kernel is ACT-bound in attention (exp is 1 elem/cycle)
and PE-bound in the projections, so the trace interleaves them:
v-projection first (produces v_aug), then per head pair
[kT chunk -> qT slice -> scores/exp/ctx], which keeps both PE and ACT
near-saturated. The PE stream is software-pipelined: scores(kc+1)
issues before ctx(kc) so PE never stalls on the current exp.
"""

import os
import sys

import numpy as np

for _p in ("/opt/trn_rl_repo", "/root/.axon_site/_ro/trn_rl_repo"):
    if os.path.isdir(_p) and _p not in sys.path:
        sys.path.insert(0, _p)

import ml_dtypes  # noqa: E402

B, S, D, H = 4, 2048, 1024, 16
HD = D // H  # 64
SQ = S // 2  # queries per core
N_CORES = 8
P = 128  # SBUF partitions
DC = D // P  # 8 chunks of the feature dim
KC = S // P  # 16 key chunks
NB = 512  # matmul moving free-dim (one PSUM bank of f32)

_BUILD_CACHE = {}


def _build_nc():
    from concourse import bacc, mybir, tile

    f32 = mybir.dt.float32
    bf16 = mybir.dt.bfloat16
    Exp = mybir.ActivationFunctionType.Exp

    nc = bacc.Bacc("TRN2", target_bir_lowering=False, debug=False, num_devices=N_CORES)

    qt_d = nc.dram_tensor("qt", [D, SQ], bf16, kind="ExternalInput")
    qlev_d = nc.dram_tensor("qlev", [D, SQ], bf16, kind="ExternalInput")
    kt_d = nc.dram_tensor("kt", [D, S], bf16, kind="ExternalInput")
    klev_d = nc.dram_tensor("klev", [D, S], bf16, kind="ExternalInput")
    vt_d = nc.dram_tensor("vt", [D, S], bf16, kind="ExternalInput")
    vlev_d = nc.dram_tensor("vlev", [S, D], bf16, kind="ExternalInput")
    wq_d = nc.dram_tensor("wq", [D, D], bf16, kind="ExternalInput")
    wk_d = nc.dram_tensor("wk", [D, D], bf16, kind="ExternalInput")
    wv_d = nc.dram_tensor("wv", [D, D], bf16, kind="ExternalInput")
    wo_d = nc.dram_tensor("wo", [D, D], bf16, kind="ExternalInput")
    bo_d = nc.dram_tensor("bo_rep", [P, D], bf16, kind="ExternalInput")
    ones16_d = nc.dram_tensor("ones16", [H, D], bf16, kind="ExternalInput")
    out_d = nc.dram_tensor("out", [SQ, D], f32, kind="ExternalOutput")

    with tile.TileContext(nc) as tc:
        with (
            tc.tile_pool(name="persist", bufs=1) as persist,
            tc.tile_pool(name="wpool", bufs=16) as wpool,
            tc.tile_pool(name="w3", bufs=1) as w3p,
            tc.tile_pool(name="kinp", bufs=8) as kinp,
            tc.tile_pool(name="qinp", bufs=8) as qinp,
            tc.tile_pool(name="vinp", bufs=12) as vinp,
            tc.tile_pool(name="lev", bufs=2) as levp,
            tc.tile_pool(name="probs", bufs=4) as prp,
            tc.tile_pool(name="norm", bufs=1) as nrm,
            tc.tile_pool(name="stgp", bufs=1) as stgp,
            tc.tile_pool(name="outp", bufs=1) as outp,
            tc.tile_pool(name="psum", bufs=1, space="PSUM") as psum,
        ):
            # Persistent intermediates (bf16).
            qT = [persist.tile([P, SQ], bf16, name=f"qT{i}", tag=f"qT{i}") for i in range(DC)]
            kT = [persist.tile([P, S], bf16, name=f"kT{i}", tag=f"kT{i}") for i in range(DC)]
            vaug = [
                persist.tile([P, H, HD + 1], bf16, name=f"vaug{i}", tag=f"vaug{i}")
                for i in range(KC)
            ]
            ctxT = [persist.tile([P, SQ], bf16, name=f"ctxT{i}", tag=f"ctxT{i}") for i in range(DC)]
            # Block-diagonal ones [16, D]: ones16[h, m] = 1 iff m // 64 == h.
            # Broadcasts the per-(head, q) reciprocal across the 64 head-dim
            # partitions via a K=16 matmul. Host-built (engine APs may only
            # start at partitions 0/32/64/96).
            ones16 = persist.tile([H, D], bf16, name="ones16", tag="ones16")

            # ---- weight/input loads (wv+wk now; wq reuses wv's slots after
            # the v projection, wo reuses wk's after the kT projection) ----
            wv_sb = [wpool.tile([P, D], bf16, name=f"wv{i}", tag="w") for i in range(DC)]
            wk_sb = [wpool.tile([P, D], bf16, name=f"wk{i}", tag="w") for i in range(DC)]
            for i in range(DC):
                nc.sync.dma_start(wv_sb[i][:], wv_d[i * P : (i + 1) * P, :])
            nc.scalar.dma_start(ones16[:], ones16_d[:])
            # kT inputs + weights go on the (early-idle) scalar DMA queue so
            # the sync queue serves the v projection's inputs first.
            kin = []
            for kc in range(DC):
                t = kinp.tile([P, S], bf16, name="kin", tag="kin")
                nc.scalar.dma_start(t[:], kt_d[kc * P : (kc + 1) * P, :])
                kin.append(t)
            for i in range(DC):
                nc.scalar.dma_start(wk_sb[i][:], wk_d[i * P : (i + 1) * P, :])

            # ---------------- v projection, emitted as fillers -------------
            # v = V @ Wv (+ vlev, which already includes bv), written into the
            # head-strided vaug layout with ones columns. Woven into head
            # pair 0's attention kc loop: vaug[m] completes one kc iteration
            # before ctx consumes it.
            vin = {}
            vstate = {}

            def v_chunk_fillers(m):
                c = m // 4
                fillers = []
                for n in range(D // NB):
                    for kc in range(DC):
                        def mmf(n=n, kc=kc, m=m, c=c):
                            if n == 0 and kc == 0 and m % 4 == 0:
                                for k2 in range(DC):
                                    t = vinp.tile([P, NB], bf16, name="vin", tag="vin")
                                    nc.sync.dma_start(
                                        t[:],
                                        vt_d[k2 * P : (k2 + 1) * P, c * NB : (c + 1) * NB],
                                    )
                                    vin[k2, c] = t
                            if kc == 0:  # noqa
                                vstate[n] = psum.tile(
                                    [P, NB], f32, name="psv", tag="ps_proj", bufs=2
                                )
                            nc.tensor.matmul(
                                vstate[n][:],
                                vin[kc, c][:, (m % 4) * P : (m % 4 + 1) * P],
                                wv_sb[kc][:, n * NB : (n + 1) * NB],
                                start=(kc == 0),
                                stop=(kc == DC - 1),
                            )
                            if kc == DC - 1:
                                lev = levp.tile([P, NB], bf16, name="levv", tag="lev")
                                nc.gpsimd.dma_start(
                                    lev[:],
                                    vlev_d[m * P : (m + 1) * P, n * NB : (n + 1) * NB],
                                )
                                hpb = NB // HD  # 8 heads per 512-col block
                                nc.vector.tensor_add(
                                    vaug[m][:, n * hpb : (n + 1) * hpb, 0:HD],
                                    vstate[n][:].rearrange("p (h d) -> p h d", h=hpb),
                                    lev[:].rearrange("p (h d) -> p h d", h=hpb),
                                )
                                if n == D // NB - 1:
                                    nc.vector.memset(vaug[m][:, :, HD : HD + 1], 1.0)
                        fillers.append(mmf)
                return fillers

            # wq reuses wv's slots; allocated lazily AFTER head pair 0's
            # attention (which carries the v projection) so the slot WAR is
            # priority-forward for the tile scheduler.
            wq_sb = []

            def load_wq():
                for i in range(DC):
                    t = wpool.tile([P, D], bf16, name=f"wq{i}", tag="w")
                    nc.sync.dma_start(t[:], wq_d[i * P : (i + 1) * P, :])
                    wq_sb.append(t)

            def kT_chunk_fillers(m):
                """Closures, one matmul each: kT[m] = Wk[:,m].T @ K.T,
                4 psum groups of 8 accumulating matmuls + DVE epilogue."""
                state = {}
                fillers = []
                for n in range(S // NB):
                    for kc in range(DC):
                        def mmf(n=n, kc=kc):
                            if kc == 0:
                                state[n] = psum.tile(
                                    [P, NB], f32, name="psk", tag="ps_proj", bufs=2
                                )
                            nc.tensor.matmul(
                                state[n][:],
                                wk_sb[kc][:, m * P : (m + 1) * P],
                                kin[kc][:, n * NB : (n + 1) * NB],
                                start=(kc == 0),
                                stop=(kc == DC - 1),
                            )
                            if kc == DC - 1:
                                lev = levp.tile([P, NB], bf16, name="levk", tag="lev")
                                nc.gpsimd.dma_start(
                                    lev[:],
                                    klev_d[m * P : (m + 1) * P, n * NB : (n + 1) * NB],
                                )
                                nc.vector.tensor_add(
                                    kT[m][:, n * NB : (n + 1) * NB], state[n][:], lev[:]
                                )
                        fillers.append(mmf)
                return fillers

            qin = {}

            def load_qin(n):
                for kc in range(DC):
                    t = qinp.tile([P, NB], bf16, name="qin", tag="qin")
                    nc.sync.dma_start(
                        t[:], qt_d[kc * P : (kc + 1) * P, n * NB : (n + 1) * NB]
                    )
                    qin[kc, n] = t

            def qT_group_fillers(m, n):
                state = {}
                fillers = []
                for kc in range(DC):
                    def mmf(kc=kc):
                        if kc == 0:
                            state[0] = psum.tile(
                                [P, NB], f32, name="psq", tag="ps_proj", bufs=2
                            )
                        nc.tensor.matmul(
                            state[0][:],
                            wq_sb[kc][:, m * P : (m + 1) * P],
                            qin[kc, n][:],
                            start=(kc == 0),
                            stop=(kc == DC - 1),
                        )
                        if kc == DC - 1:
                            lev = levp.tile([P, NB], bf16, name="levq", tag="lev")
                            nc.gpsimd.dma_start(
                                lev[:],
                                qlev_d[m * P : (m + 1) * P, n * NB : (n + 1) * NB],
                            )
                            nc.vector.tensor_add(
                                qT[m][:, n * NB : (n + 1) * NB], state[0][:], lev[:]
                            )
                    fillers.append(mmf)
                return fillers

            def run_fillers(fillers, k):
                for _ in range(min(k, len(fillers))):
                    fillers.pop(0)()

            def emit_attention(qb, hp, fillers=None, per_kc=3):
                qs = slice(qb * NB, (qb + 1) * NB)
                fillers = fillers if fillers is not None else []
                cps = [
                    psum.tile([HD + 1, NB], f32, name=f"cps{e}", tag="ctxps", bufs=2)
                    for e in range(2)
                ]
                # software pipeline per kc: scores(kc); exp(kc); PE filler
                # work (projections/outproj) in the exp-wait gap; ctx(kc-1)
                prev_pr = None
                for kc in range(KC + 1):
                    if kc < KC:
                        sps = psum.tile([P, 2 * NB], f32, name="sps", tag="sps", bufs=2)
                        for e in range(2):
                            rows = slice(e * HD, (e + 1) * HD)
                            # head pair packed in PE row halves
                            nc.tensor.matmul(
                                sps[:, e * NB : (e + 1) * NB],
                                kT[hp][rows, kc * P : (kc + 1) * P],
                                qT[hp][rows, qs],
                                start=True,
                                stop=True,
                            )
                        pr = prp.tile([P, 2 * NB], bf16, name="pr", tag="pr")
                        nc.scalar.activation(pr[:], sps[:], Exp, scale=1.0 / 8.0)
                        run_fillers(fillers, per_kc)
                    if kc > 0:
                        pkc = kc - 1
                        for e in range(2):
                            nc.tensor.matmul(
                                cps[e][:],
                                vaug[pkc][:, 2 * hp + e, :],
                                prev_pr[:, e * NB : (e + 1) * NB],
                                start=(pkc == 0),
                                stop=(pkc == KC - 1),
                            )
                    if kc < KC:
                        prev_pr = pr
                run_fillers(fillers, len(fillers))
                sums2 = nrm.tile([2, NB], f32, name="sums2", tag="sums2", bufs=2)
                for e in range(2):
                    rows = slice(e * HD, (e + 1) * HD)
                    # Stash the denominator row: engines cannot write an
                    # arbitrary partition (bases limited to 0/32/64/96), so
                    # stage on partition 64 in SBUF then DMA into sums2[e].
                    stg = stgp.tile([HD + 1, NB], f32, name="stg", tag="stg")
                    nc.vector.tensor_copy(stg[HD : HD + 1, :], cps[e][HD : HD + 1, :])
                    nc.gpsimd.dma_start(sums2[e : e + 1, :], stg[HD : HD + 1, :])
                    # copy unnormalized ctx (normalized in place later)
                    nc.vector.tensor_copy(ctxT[hp][rows, qs], cps[e][0:HD, :])
                return (qb, hp, sums2)

            def emit_norm_finish(pend):
                # Normalize a head pair (deferred one iteration so the DVE
                # queue never waits on the sums2 DMA): 1/sums broadcast
                # across the 64 head-dim partitions via a K=2 matmul against
                # the [2, 128] top-left block of ones16.
                qb, hp, sums2 = pend
                qs = slice(qb * NB, (qb + 1) * NB)
                recf2 = nrm.tile([2, NB], f32, name="recf2", tag="recf2")
                nc.vector.reciprocal_approx_fast(recf2[:], sums2[:])
                recb2 = nrm.tile([2, NB], bf16, name="recb2", tag="recb2")
                nc.vector.tensor_copy(recb2[:], recf2[:])
                bc = psum.tile([P, NB], f32, name="bc", tag="ps_proj", bufs=2)
                nc.tensor.matmul(bc[:], ones16[0:2, 0:P], recb2[:], start=True, stop=True)
                nc.vector.tensor_mul(ctxT[hp][:, qs], ctxT[hp][:, qs], bc[:])

            def outproj_fillers(qg, n):
                state = {}
                fillers = []
                for dc in range(DC):
                    def mmf(dc=dc):
                        if dc == 0:
                            state[0] = psum.tile(
                                [P, NB], f32, name="pso", tag="ps_proj", bufs=2
                            )
                        nc.tensor.matmul(
                            state[0][:],
                            ctxT[dc][:, qg * P : (qg + 1) * P],
                            wo_sb[dc][:, n * NB : (n + 1) * NB],
                            start=(dc == 0),
                            stop=(dc == DC - 1),
                        )
                        if dc == DC - 1:
                            ot = outp.tile([P, NB], f32, name="ot", tag="ot")
                            nc.vector.tensor_add(
                                ot[:], state[0][:], bo_sb[:, n * NB : (n + 1) * NB]
                            )
                            nc.sync.dma_start(
                                out_d[qg * P : (qg + 1) * P, n * NB : (n + 1) * NB],
                                ot[:],
                            )
                    fillers.append(mmf)
                return fillers

            def emit_outproj(qg, n):
                run = outproj_fillers(qg, n)
                for f in run:
                    f()

            # ---- interleaved schedule ----
            # Upfront: kT[0] and qT[0] n=0, then head pair 0's attention
            # carries the whole v projection (+ kT[1]/qT[1]) as fillers;
            # later pairs carry the next pair's kT/qT in their exp-wait gaps.
            load_qin(0)
            for f in kT_chunk_fillers(0):
                f()
            # qT[0] n=0 with a dedicated load of wq's first column block
            # (the full wq load must wait for v to release the wv slots).
            wq0 = []
            for kc in range(DC):
                t = w3p.tile([P, P], bf16, name=f"wq0_{kc}", tag=f"wq0_{kc}")
                nc.sync.dma_start(t[:], wq_d[kc * P : (kc + 1) * P, 0:P])
                wq0.append(t)
            ps00 = psum.tile([P, NB], f32, name="psq00", tag="ps_proj", bufs=2)
            for kc in range(DC):
                nc.tensor.matmul(
                    ps00[:],
                    wq0[kc][:],
                    qin[kc, 0][:],
                    start=(kc == 0),
                    stop=(kc == DC - 1),
                )
            lev00 = levp.tile([P, NB], bf16, name="levq", tag="lev")
            nc.gpsimd.dma_start(lev00[:], qlev_d[0:P, 0:NB])
            nc.vector.tensor_add(qT[0][:, 0:NB], ps00[:], lev00[:])
            with nc.named_scope("proj_v"):
                for m in range(KC):
                    for f in v_chunk_fillers(m):
                        f()
            load_wq()
            pend = None
            with nc.named_scope("qb0"):
                for hp in range(H // 2):
                    fillers = []
                    if hp + 1 < H // 2:
                        fillers += kT_chunk_fillers(hp + 1)
                        fillers += qT_group_fillers(hp + 1, 0)
                    else:
                        load_qin(1)
                        for m in range(4):
                            fillers += qT_group_fillers(m, 1)
                    per_kc = (len(fillers) + KC - 1) // KC
                    nxt = emit_attention(0, hp, fillers, per_kc=per_kc)
                    if pend is not None:
                        emit_norm_finish(pend)
                    pend = nxt

            # wo reuses wk's slots (kT projection is done by now)
            wo_sb = [wpool.tile([P, D], bf16, name=f"wo{i}", tag="w") for i in range(DC)]
            for i in range(DC):
                nc.sync.dma_start(wo_sb[i][:], wo_d[i * P : (i + 1) * P, :])
            bo_sb = w3p.tile([P, D], bf16, name="bo_sb", tag="bo_sb")
            nc.sync.dma_start(bo_sb[:], bo_d[:])

            # qb1: remaining qT n=1 groups, then qb0's output projection
            # (q-chunks 0-3) interleave with the ACT-bound attention.
            op0 = [(qg, n) for qg in range(NB // P) for n in range(D // NB)]
            opq = list(op0)  # qb0's 8 outproj groups, spread over hp 1-7
            with nc.named_scope("qb1"):
                for hp in range(H // 2):
                    fillers = []
                    if hp == 0:
                        for m in range(4, DC):
                            fillers += qT_group_fillers(m, 1)
                    else:
                        take = 2 if hp == 7 else 1
                        for _ in range(take):
                            if opq:
                                fillers += outproj_fillers(*opq.pop(0))
                    nxt = emit_attention(1, hp, fillers)
                    emit_norm_finish(pend)
                    pend = nxt
            while opq:
                emit_outproj(*opq.pop(0))
            emit_norm_finish(pend)
            with nc.named_scope("outproj_tail"):
                for qg in range(NB // P, SQ // P):
                    for n in range(D // NB):
                        emit_outproj(qg, n)

    nc.compile()
    return nc


def get_nc():
    if "nc" not in _BUILD_CACHE:
        _BUILD_CACHE["nc"] = _build_nc()
    return _BUILD_CACHE["nc"]


def make_in_maps(inputs):
    bf16 = ml_dtypes.bfloat16
    f32 = np.float32
    Q = np.asarray(inputs["Q"], f32)
    Q_lev = np.asarray(inputs["Q_lev"], f32)
    K = np.asarray(inputs["K"], f32)
    K_lev = np.asarray(inputs["K_lev"], f32)
    V = np.asarray(inputs["V"], f32)
    V_lev = np.asarray(inputs["V_lev"], f32)
    bq = np.asarray(inputs["bq"], f32)
    bk = np.asarray(inputs["bk"], f32)
    bv = np.asarray(inputs["bv"], f32)
    bo = np.asarray(inputs["bo"], f32)

    shared = {
        "wq": np.ascontiguousarray(np.asarray(inputs["Wq"], f32).astype(bf16)),
        "wk": np.ascontiguousarray(np.asarray(inputs["Wk"], f32).astype(bf16)),
        "wv": np.ascontiguousarray(np.asarray(inputs["Wv"], f32).astype(bf16)),
        "wo": np.ascontiguousarray(np.asarray(inputs["Wo"], f32).astype(bf16)),
        "bo_rep": np.ascontiguousarray(np.tile(bo.reshape(1, -1), (P, 1))).astype(bf16),
        "ones16": np.kron(np.eye(H, dtype=f32), np.ones((1, HD), f32)).astype(bf16),
    }
    per_batch = []
    for b in range(B):
        per_batch.append(
            {
                "kt": np.ascontiguousarray(K[b].T.astype(bf16)),
                "klev": np.ascontiguousarray((K_lev[b] + bk).T).astype(bf16),
                "vt": np.ascontiguousarray(V[b].T.astype(bf16)),
                "vlev": np.ascontiguousarray(V_lev[b] + bv).astype(bf16),
            }
        )
    in_maps = []
    for c in range(N_CORES):
        b, hf = divmod(c, 2)
        qs = slice(hf * SQ, (hf + 1) * SQ)
        in_maps.append(
            {
                "qt": np.ascontiguousarray(Q[b, qs, :].T.astype(bf16)),
                "qlev": np.ascontiguousarray((Q_lev[b, qs, :] + bq).T).astype(bf16),
                **per_batch[b],
                **shared,
            }
        )
    return in_maps


def run_on_cores(inputs, trace=False):
    """Run the SPMD kernel; returns (full_output, BassKernelResults)."""
    from concourse.bass_utils import run_bass_kernel_spmd

    nc = get_nc()
    in_maps = make_in_maps(inputs)
    res = run_bass_kernel_spmd(nc, in_maps, core_ids=list(range(N_CORES)), trace=trace)
    out = np.empty((B, S, D), np.float32)
    for c in range(N_CORES):
        b, hf = divmod(c, 2)
        out[b, hf * SQ : (hf + 1) * SQ, :] = res.results[c]["out"]
    return out, res


def kernel(**inputs):
    out, _ = run_on_cores(inputs, trace=False)
    return out


if __name__ == "__main__":
    nc = get_nc()
    print("built + compiled OK")


# revision 31
# speedup vs baseline: 1.1035x; 1.1035x over previous
"""Distributed Trainium2 Bass kernel for multi-head attention.

Reference computation (B=4, S=2048, D=1024, H=16 heads, HD=64):
    q = heads(Q @ Wq + bq + Q_lev)
    k = heads(K @ Wk + bk + K_lev)
    v = heads(V @ Wv + bv + V_lev)
    out = softmax(q k^T / sqrt(HD)) v  -> merge heads -> @ Wo + bo

Sharding: 8 cores = 4 batches x 2 query-halves (1024 queries each).
Each core computes its [1024, 1024] output slice end-to-end with zero
collectives; the K/V projections are recomputed by both cores of a
batch pair (cheaper than any 2-rank collective on this chip).

Device-side layout strategy (everything feature-major / pre-transposed
on the host so no on-chip transposes are needed):
  qT   [D, Sq]  = Wq.T @ Q.T   (+ bq + Q_lev, folded on host into qlevT)
  kT   [D, S]   = Wk.T @ K.T
  v    [S, D]   = V @ Wv       (stationary = V.T, moving = Wv)
  scoresT[keys, q] = kT_h.T @ qT_h        (contract over HD=64)
  probsT = exp(scoresT / 8)               (no max subtraction: scores are
                                           N(0,~2) so exp stays < ~1e6)
  ctxT_aug[65, q] = v_aug_h.T @ probsT    (v_aug has a 65th ones column,
                                           so row 64 = softmax denominator)
  ctxT = ctxT_aug[:64] * (1/denominator)  (batched reciprocal + block-diag
                                           ones matmul to broadcast 1/sum
                                           across the 64 head-dim partitions)
  out[q, D] = ctxT.T @ Wo (+ bo)

Matmuls run in bf16 (f32 PSUM accumulation). The two K=64 scores
matmuls of a head pair run concurrently in PE row halves (tile_position
auto-derived from base partitions 0/64) and write the two banks of one
[128, 1024] PSUM tile so a single wide ACT exp serves both heads.

Scheduling: the kernel is ACT-bound in attention (exp is 1 elem/cycle)
and PE-bound in the projections, so the trace interleaves them:
v-projection first (produces v_aug), then per head pair
[kT chunk -> qT slice -> scores/exp/ctx], which keeps both PE and ACT
near-saturated. The PE stream is software-pipelined: scores(kc+1)
issues before ctx(kc) so PE never stalls on the current exp.
"""

import os
import sys

import numpy as np

for _p in ("/opt/trn_rl_repo", "/root/.axon_site/_ro/trn_rl_repo"):
    if os.path.isdir(_p) and _p not in sys.path:
        sys.path.insert(0, _p)

import ml_dtypes  # noqa: E402

B, S, D, H = 4, 2048, 1024, 16
HD = D // H  # 64
SQ = S // 2  # queries per core
N_CORES = 8
P = 128  # SBUF partitions
DC = D // P  # 8 chunks of the feature dim
KC = S // P  # 16 key chunks
NB = 512  # matmul moving free-dim (one PSUM bank of f32)

_BUILD_CACHE = {}


def _build_nc():
    from concourse import bacc, mybir, tile

    f32 = mybir.dt.float32
    bf16 = mybir.dt.bfloat16
    Exp = mybir.ActivationFunctionType.Exp

    nc = bacc.Bacc("TRN2", target_bir_lowering=False, debug=False, num_devices=N_CORES)

    qt_d = nc.dram_tensor("qt", [D, SQ], bf16, kind="ExternalInput")
    qlev_d = nc.dram_tensor("qlev", [D, SQ], bf16, kind="ExternalInput")
    kt_d = nc.dram_tensor("kt", [D, S], bf16, kind="ExternalInput")
    klev_d = nc.dram_tensor("klev", [D, S], bf16, kind="ExternalInput")
    vt_d = nc.dram_tensor("vt", [D, S], bf16, kind="ExternalInput")
    vlev_d = nc.dram_tensor("vlev", [S, D], bf16, kind="ExternalInput")
    wq_d = nc.dram_tensor("wq", [D, D], bf16, kind="ExternalInput")
    wk_d = nc.dram_tensor("wk", [D, D], bf16, kind="ExternalInput")
    wv_d = nc.dram_tensor("wv", [D, D], bf16, kind="ExternalInput")
    wo_d = nc.dram_tensor("wo", [D, D], bf16, kind="ExternalInput")
    bo_d = nc.dram_tensor("bo_rep", [P, D], bf16, kind="ExternalInput")
    ones16_d = nc.dram_tensor("ones16", [H, D], bf16, kind="ExternalInput")
    out_d = nc.dram_tensor("out", [SQ, D], f32, kind="ExternalOutput")

    with tile.TileContext(nc) as tc:
        with (
            tc.tile_pool(name="persist", bufs=1) as persist,
            tc.tile_pool(name="wpool", bufs=16) as wpool,
            tc.tile_pool(name="w3", bufs=1) as w3p,
            tc.tile_pool(name="kinp", bufs=8) as kinp,
            tc.tile_pool(name="qinp", bufs=8) as qinp,
            tc.tile_pool(name="vinp", bufs=12) as vinp,
            tc.tile_pool(name="lev", bufs=2) as levp,
            tc.tile_pool(name="probs", bufs=4) as prp,
            tc.tile_pool(name="norm", bufs=1) as nrm,
            tc.tile_pool(name="stgp", bufs=1) as stgp,
            tc.tile_pool(name="outp", bufs=1) as outp,
            tc.tile_pool(name="psum", bufs=1, space="PSUM") as psum,
        ):
            # Persistent intermediates (bf16).
            qT = [persist.tile([P, SQ], bf16, name=f"qT{i}", tag=f"qT{i}") for i in range(DC)]
            kT = [persist.tile([P, S], bf16, name=f"kT{i}", tag=f"kT{i}") for i in range(DC)]
            vaug = [
                persist.tile([P, H, HD + 1], bf16, name=f"vaug{i}", tag=f"vaug{i}")
                for i in range(KC)
            ]
            ctxT = [persist.tile([P, SQ], bf16, name=f"ctxT{i}", tag=f"ctxT{i}") for i in range(DC)]
            # Block-diagonal ones [16, D]: ones16[h, m] = 1 iff m // 64 == h.
            # Broadcasts the per-(head, q) reciprocal across the 64 head-dim
            # partitions via a K=16 matmul. Host-built (engine APs may only
            # start at partitions 0/32/64/96).
            ones16 = persist.tile([H, D], bf16, name="ones16", tag="ones16")

            # ---- weight/input loads (wv+wk now; wq reuses wv's slots after
            # the v projection, wo reuses wk's after the kT projection) ----
            wv_sb = [wpool.tile([P, D], bf16, name=f"wv{i}", tag="w") for i in range(DC)]
            wk_sb = [wpool.tile([P, D], bf16, name=f"wk{i}", tag="w") for i in range(DC)]
            for i in range(DC):
                nc.sync.dma_start(wv_sb[i][:], wv_d[i * P : (i + 1) * P, :])
            nc.scalar.dma_start(ones16[:], ones16_d[:])
            # kT inputs + weights go on the (early-idle) scalar DMA queue so
            # the sync queue serves the v projection's inputs first.
            kin = []
            for kc in range(DC):
                t = kinp.tile([P, S], bf16, name="kin", tag="kin")
                nc.scalar.dma_start(t[:], kt_d[kc * P : (kc + 1) * P, :])
                kin.append(t)
            for i in range(DC):
                nc.scalar.dma_start(wk_sb[i][:], wk_d[i * P : (i + 1) * P, :])

            # ---------------- v projection, emitted as fillers -------------
            # v = V @ Wv (+ vlev, which already includes bv), written into the
            # head-strided vaug layout with ones columns. Woven into head
            # pair 0's attention kc loop: vaug[m] completes one kc iteration
            # before ctx consumes it.
            vin = {}
            vstate = {}

            def v_chunk_fillers(m):
                c = m // 4
                fillers = []
                for n in range(D // NB):
                    for kc in range(DC):
                        def mmf(n=n, kc=kc, m=m, c=c):
                            if n == 0 and kc == 0 and m % 4 == 0:
                                for k2 in range(DC):
                                    t = vinp.tile([P, NB], bf16, name="vin", tag="vin")
                                    nc.sync.dma_start(
                                        t[:],
                                        vt_d[k2 * P : (k2 + 1) * P, c * NB : (c + 1) * NB],
                                    )
                                    vin[k2, c] = t
                            if kc == 0:  # noqa
                                vstate[n] = psum.tile(
                                    [P, NB], f32, name="psv", tag="ps_proj", bufs=2
                                )
                            nc.tensor.matmul(
                                vstate[n][:],
                                vin[kc, c][:, (m % 4) * P : (m % 4 + 1) * P],
                                wv_sb[kc][:, n * NB : (n + 1) * NB],
                                start=(kc == 0),
                                stop=(kc == DC - 1),
                            )
                            if kc == DC - 1:
                                lev = levp.tile([P, NB], bf16, name="levv", tag="lev")
                                nc.gpsimd.dma_start(
                                    lev[:],
                                    vlev_d[m * P : (m + 1) * P, n * NB : (n + 1) * NB],
                                )
                                hpb = NB // HD  # 8 heads per 512-col block
                                nc.vector.tensor_add(
                                    vaug[m][:, n * hpb : (n + 1) * hpb, 0:HD],
                                    vstate[n][:].rearrange("p (h d) -> p h d", h=hpb),
                                    lev[:].rearrange("p (h d) -> p h d", h=hpb),
                                )
                                if n == D // NB - 1:
                                    nc.vector.memset(vaug[m][:, :, HD : HD + 1], 1.0)
                        fillers.append(mmf)
                return fillers

            # wq reuses wv's slots; allocated lazily AFTER head pair 0's
            # attention (which carries the v projection) so the slot WAR is
            # priority-forward for the tile scheduler.
            wq_sb = []

            def load_wq():
                for i in range(DC):
                    t = wpool.tile([P, D], bf16, name=f"wq{i}", tag="w")
                    nc.sync.dma_start(t[:], wq_d[i * P : (i + 1) * P, :])
                    wq_sb.append(t)

            def kT_chunk_fillers(m):
                """Closures, one matmul each: kT[m] = Wk[:,m].T @ K.T,
                4 psum groups of 8 accumulating matmuls + DVE epilogue."""
                state = {}
                fillers = []
                for n in range(S // NB):
                    for kc in range(DC):
                        def mmf(n=n, kc=kc):
                            if kc == 0:
                                state[n] = psum.tile(
                                    [P, NB], f32, name="psk", tag="ps_proj", bufs=2
                                )
                            nc.tensor.matmul(
                                state[n][:],
                                wk_sb[kc][:, m * P : (m + 1) * P],
                                kin[kc][:, n * NB : (n + 1) * NB],
                                start=(kc == 0),
                                stop=(kc == DC - 1),
                            )
                            if kc == DC - 1:
                                lev = levp.tile([P, NB], bf16, name="levk", tag="lev")
                                nc.gpsimd.dma_start(
                                    lev[:],
                                    klev_d[m * P : (m + 1) * P, n * NB : (n + 1) * NB],
                                )
                                nc.vector.tensor_add(
                                    kT[m][:, n * NB : (n + 1) * NB], state[n][:], lev[:]
                                )
                        fillers.append(mmf)
                return fillers

            qin = {}

            def load_qin(n):
                for kc in range(DC):
                    t = qinp.tile([P, NB], bf16, name="qin", tag="qin")
                    nc.sync.dma_start(
                        t[:], qt_d[kc * P : (kc + 1) * P, n * NB : (n + 1) * NB]
                    )
                    qin[kc, n] = t

            def qT_group_fillers(m, n):
                state = {}
                fillers = []
                for kc in range(DC):
                    def mmf(kc=kc):
                        if kc == 0:
                            state[0] = psum.tile(
                                [P, NB], f32, name="psq", tag="ps_proj", bufs=2
                            )
                        nc.tensor.matmul(
                            state[0][:],
                            wq_sb[kc][:, m * P : (m + 1) * P],
                            qin[kc, n][:],
                            start=(kc == 0),
                            stop=(kc == DC - 1),
                        )
                        if kc == DC - 1:
                            lev = levp.tile([P, NB], bf16, name="levq", tag="lev")
                            nc.gpsimd.dma_start(
                                lev[:],
                                qlev_d[m * P : (m + 1) * P, n * NB : (n + 1) * NB],
                            )
                            nc.vector.tensor_add(
                                qT[m][:, n * NB : (n + 1) * NB], state[0][:], lev[:]
                            )
                    fillers.append(mmf)
                return fillers

            def run_fillers(fillers, k):
                for _ in range(min(k, len(fillers))):
                    fillers.pop(0)()

            def emit_attention(qb, hp, fillers=None, per_kc=3):
                qs = slice(qb * NB, (qb + 1) * NB)
                fillers = fillers if fillers is not None else []
                cps = [
                    psum.tile([HD + 1, NB], f32, name=f"cps{e}", tag="ctxps", bufs=2)
                    for e in range(2)
                ]
                # software pipeline per kc: scores(kc); exp(kc); PE filler
                # work (projections/outproj) in the exp-wait gap; ctx(kc-1)
                prev_pr = None
                for kc in range(KC + 1):
                    if kc < KC:
                        sps = psum.tile([P, 2 * NB], f32, name="sps", tag="sps", bufs=2)
                        for e in range(2):
                            rows = slice(e * HD, (e + 1) * HD)
                            # head pair packed in PE row halves
                            nc.tensor.matmul(
                                sps[:, e * NB : (e + 1) * NB],
                                kT[hp][rows, kc * P : (kc + 1) * P],
                                qT[hp][rows, qs],
                                start=True,
                                stop=True,
                            )
                        pr = prp.tile([P, 2 * NB], bf16, name="pr", tag="pr")
                        nc.scalar.activation(pr[:], sps[:], Exp, scale=1.0 / 8.0)
                        run_fillers(fillers, per_kc)
                    if kc > 0:
                        pkc = kc - 1
                        for e in range(2):
                            nc.tensor.matmul(
                                cps[e][:],
                                vaug[pkc][:, 2 * hp + e, :],
                                prev_pr[:, e * NB : (e + 1) * NB],
                                start=(pkc == 0),
                                stop=(pkc == KC - 1),
                            )
                    if kc < KC:
                        prev_pr = pr
                run_fillers(fillers, len(fillers))
                sums2 = nrm.tile([2, NB], f32, name="sums2", tag="sums2", bufs=2)
                for e in range(2):
                    rows = slice(e * HD, (e + 1) * HD)
                    # Stash the denominator row: engines cannot write an
                    # arbitrary partition (bases limited to 0/32/64/96), so
                    # stage on partition 64 in SBUF then DMA into sums2[e].
                    stg = stgp.tile([HD + 1, NB], f32, name="stg", tag="stg")
                    nc.vector.tensor_copy(stg[HD : HD + 1, :], cps[e][HD : HD + 1, :])
                    nc.gpsimd.dma_start(sums2[e : e + 1, :], stg[HD : HD + 1, :])
                    # copy unnormalized ctx (normalized in place later)
                    nc.vector.tensor_copy(ctxT[hp][rows, qs], cps[e][0:HD, :])
                return (qb, hp, sums2)

            def emit_norm_finish(pend):
                # Normalize a head pair (deferred one iteration so the DVE
                # queue never waits on the sums2 DMA): 1/sums broadcast
                # across the 64 head-dim partitions via a K=2 matmul against
                # the [2, 128] top-left block of ones16.
                qb, hp, sums2 = pend
                qs = slice(qb * NB, (qb + 1) * NB)
                recf2 = nrm.tile([2, NB], f32, name="recf2", tag="recf2")
                nc.vector.reciprocal_approx_fast(recf2[:], sums2[:])
                recb2 = nrm.tile([2, NB], bf16, name="recb2", tag="recb2")
                nc.vector.tensor_copy(recb2[:], recf2[:])
                bc = psum.tile([P, NB], f32, name="bc", tag="ps_proj", bufs=2)
                nc.tensor.matmul(bc[:], ones16[0:2, 0:P], recb2[:], start=True, stop=True)
                nc.vector.tensor_mul(ctxT[hp][:, qs], ctxT[hp][:, qs], bc[:])

            def outproj_fillers(qg, n):
                state = {}
                fillers = []
                for dc in range(DC):
                    def mmf(dc=dc):
                        if dc == 0:
                            state[0] = psum.tile(
                                [P, NB], f32, name="pso", tag="ps_proj", bufs=2
                            )
                        nc.tensor.matmul(
                            state[0][:],
                            ctxT[dc][:, qg * P : (qg + 1) * P],
                            wo_sb[dc][:, n * NB : (n + 1) * NB],
                            start=(dc == 0),
                            stop=(dc == DC - 1),
                        )
                        if dc == DC - 1:
                            ot = outp.tile([P, NB], f32, name="ot", tag="ot")
                            nc.vector.tensor_add(
                                ot[:], state[0][:], bo_sb[:, n * NB : (n + 1) * NB]
                            )
                            nc.sync.dma_start(
                                out_d[qg * P : (qg + 1) * P, n * NB : (n + 1) * NB],
                                ot[:],
                            )
                    fillers.append(mmf)
                return fillers

            def emit_outproj(qg, n):
                run = outproj_fillers(qg, n)
                for f in run:
                    f()

            # ---- interleaved schedule ----
            # Upfront: kT[0] and qT[0] n=0, then head pair 0's attention
            # carries the whole v projection (+ kT[1]/qT[1]) as fillers;
            # later pairs carry the next pair's kT/qT in their exp-wait gaps.
            load_qin(0)
            for f in kT_chunk_fillers(0):
                f()
            # qT[0] n=0 with a dedicated load of wq's first column block
            # (the full wq load must wait for v to release the wv slots).
            wq0 = []
            for kc in range(DC):
                t = w3p.tile([P, P], bf16, name=f"wq0_{kc}", tag=f"wq0_{kc}")
                nc.sync.dma_start(t[:], wq_d[kc * P : (kc + 1) * P, 0:P])
                wq0.append(t)
            ps00 = psum.tile([P, NB], f32, name="psq00", tag="ps_proj", bufs=2)
            for kc in range(DC):
                nc.tensor.matmul(
                    ps00[:],
                    wq0[kc][:],
                    qin[kc, 0][:],
                    start=(kc == 0),
                    stop=(kc == DC - 1),
                )
            lev00 = levp.tile([P, NB], bf16, name="levq", tag="lev")
            nc.gpsimd.dma_start(lev00[:], qlev_d[0:P, 0:NB])
            nc.vector.tensor_add(qT[0][:, 0:NB], ps00[:], lev00[:])
            with nc.named_scope("proj_v"):
                for m in range(KC):
                    for f in v_chunk_fillers(m):
                        f()
            load_wq()
            pend = None
            with nc.named_scope("qb0"):
                for hp in range(H // 2):
                    fillers = []
                    if hp + 1 < H // 2:
                        fillers += kT_chunk_fillers(hp + 1)
                        fillers += qT_group_fillers(hp + 1, 0)
                    else:
                        load_qin(1)
                        for m in range(4):
                            fillers += qT_group_fillers(m, 1)
                    per_kc = (len(fillers) + KC - 1) // KC
                    nxt = emit_attention(0, hp, fillers, per_kc=per_kc)
                    if pend is not None:
                        emit_norm_finish(pend)
                    pend = nxt

            # wo reuses wk's slots (kT projection is done by now)
            wo_sb = [wpool.tile([P, D], bf16, name=f"wo{i}", tag="w") for i in range(DC)]
            for i in range(DC):
                nc.sync.dma_start(wo_sb[i][:], wo_d[i * P : (i + 1) * P, :])
            bo_sb = w3p.tile([P, D], bf16, name="bo_sb", tag="bo_sb")
            nc.sync.dma_start(bo_sb[:], bo_d[:])

            # qb1: remaining qT n=1 groups, then qb0's output projection
            # (q-chunks 0-3) interleave with the ACT-bound attention.
            op0 = [(qg, n) for qg in range(NB // P) for n in range(D // NB)]
            opq = list(op0)  # qb0's 8 outproj groups, two per pair hp 1-4
            with nc.named_scope("qb1"):
                for hp in range(H // 2):
                    fillers = []
                    if hp == 0:
                        for m in range(4, DC):
                            fillers += qT_group_fillers(m, 1)
                    elif hp <= 4:
                        fillers += outproj_fillers(*opq.pop(0))
                        fillers += outproj_fillers(*opq.pop(0))
                    nxt = emit_attention(1, hp, fillers)
                    emit_norm_finish(pend)
                    pend = nxt
            while opq:
                emit_outproj(*opq.pop(0))
            emit_norm_finish(pend)
            with nc.named_scope("outproj_tail"):
                for qg in range(NB // P, SQ // P):
                    for n in range(D // NB):
                        emit_outproj(qg, n)

    nc.compile()
    return nc


def get_nc():
    if "nc" not in _BUILD_CACHE:
        _BUILD_CACHE["nc"] = _build_nc()
    return _BUILD_CACHE["nc"]


def make_in_maps(inputs):
    bf16 = ml_dtypes.bfloat16
    f32 = np.float32
    Q = np.asarray(inputs["Q"], f32)
    Q_lev = np.asarray(inputs["Q_lev"], f32)
    K = np.asarray(inputs["K"], f32)
    K_lev = np.asarray(inputs["K_lev"], f32)
    V = np.asarray(inputs["V"], f32)
    V_lev = np.asarray(inputs["V_lev"], f32)
    bq = np.asarray(inputs["bq"], f32)
    bk = np.asarray(inputs["bk"], f32)
    bv = np.asarray(inputs["bv"], f32)
    bo = np.asarray(inputs["bo"], f32)

    shared = {
        "wq": np.ascontiguousarray(np.asarray(inputs["Wq"], f32).astype(bf16)),
        "wk": np.ascontiguousarray(np.asarray(inputs["Wk"], f32).astype(bf16)),
        "wv": np.ascontiguousarray(np.asarray(inputs["Wv"], f32).astype(bf16)),
        "wo": np.ascontiguousarray(np.asarray(inputs["Wo"], f32).astype(bf16)),
        "bo_rep": np.ascontiguousarray(np.tile(bo.reshape(1, -1), (P, 1))).astype(bf16),
        "ones16": np.kron(np.eye(H, dtype=f32), np.ones((1, HD), f32)).astype(bf16),
    }
    per_batch = []
    for b in range(B):
        per_batch.append(
            {
                "kt": np.ascontiguousarray(K[b].T.astype(bf16)),
                "klev": np.ascontiguousarray((K_lev[b] + bk).T).astype(bf16),
                "vt": np.ascontiguousarray(V[b].T.astype(bf16)),
                "vlev": np.ascontiguousarray(V_lev[b] + bv).astype(bf16),
            }
        )
    in_maps = []
    for c in range(N_CORES):
        b, hf = divmod(c, 2)
        qs = slice(hf * SQ, (hf + 1) * SQ)
        in_maps.append(
            {
                "qt": np.ascontiguousarray(Q[b, qs, :].T.astype(bf16)),
                "qlev": np.ascontiguousarray((Q_lev[b, qs, :] + bq).T).astype(bf16),
                **per_batch[b],
                **shared,
            }
        )
    return in_maps


def run_on_cores(inputs, trace=False):
    """Run the SPMD kernel; returns (full_output, BassKernelResults)."""
    from concourse.bass_utils import run_bass_kernel_spmd

    nc = get_nc()
    in_maps = make_in_maps(inputs)
    res = run_bass_kernel_spmd(nc, in_maps, core_ids=list(range(N_CORES)), trace=trace)
    out = np.empty((B, S, D), np.float32)
    for c in range(N_CORES):
        b, hf = divmod(c, 2)
        out[b, hf * SQ : (hf + 1) * SQ, :] = res.results[c]["out"]
    return out, res


def kernel(**inputs):
    out, _ = run_on_cores(inputs, trace=False)
    return out


if __name__ == "__main__":
    nc = get_nc()
    print("built + compiled OK")
